# revision 4
# baseline (speedup 1.0000x reference)
"""DeepseekMoE (moe_routing) Trainium2 kernel.

Strategy (8 NeuronCores, single SPMD program):
  - Routing (grouped top-k; tiny T x H @ H x E) runs on host in numpy.
  - Routed experts are expert-parallel: each core owns 2 experts (one
    "big" slot, one "small" slot; capacities sized at call time from the
    actual per-expert token counts, so the compiled shapes adapt to the
    data). Tokens for each expert are gathered host-side into a
    transposed [H, C] activation block per slot; the device runs
    grouped GEMM1 -> SwiGLU -> GEMM2 per slot with the top-k combine
    weight folded into the GEMM2 PSUM evict.
  - Shared expert MLP is tensor-parallel over the 8 cores along the
    intermediate dim (2816 -> 8 x 352, zero-padded to 8 x 384).
  - All matmul operands are bf16 (cast host-side), accumulation f32.
  - Device returns per-slot y^T [H, C] plus the shared partial [H, T];
    host sums partials and scatter-adds slot outputs.
"""

import numpy as np
import ml_dtypes

import concourse.mybir as mybir
import concourse.tile as tile
from concourse import bacc
from concourse.bass_utils import run_bass_kernel_spmd

BF16 = ml_dtypes.bfloat16
F32 = np.float32

# Problem shapes (fixed by the spec).
T, H, E, I = 1024, 2048, 16, 1408
I2 = 2 * I                      # 2816 (w13 rows per expert)
IS = 2 * I                      # shared intermediate (n_shared=2 -> 2816)
SSH = 384                       # per-core shared shard (2816 padded to 3072 = 8*384)
TOP_K, N_GROUP, TOPK_GROUP = 4, 4, 2
ROUTED_SCALE = 2.5
N_CORES = 8
P = 128
KH = H // P                     # 16 K-subtiles over H
KI = I // P                     # 11 K-subtiles over I
MW = I2 // P                    # 22 M-panels over 2I
MH = H // P                     # 16 M-panels over H
NPAIR = I // P                  # 11 (g,u) SwiGLU pairs per slot
KS = SSH // P                   # 3 K-subtiles over shared shard


def _sigmoid(x):
    return 1.0 / (1.0 + np.exp(-x))


def _route(x, gate_weight, gate_bias):
    """Numpy port of reference._grouped_topk (float64 internally)."""
    logits = x.astype(np.float64) @ gate_weight.astype(np.float64).T
    scores = _sigmoid(logits)
    choice = scores + gate_bias.astype(np.float64)[None, :]
    g = choice.reshape(T, N_GROUP, E // N_GROUP)
    top2sum = np.sort(g, axis=-1)[..., -2:].sum(-1)          # [T, NG]
    gidx = np.argsort(-top2sum, axis=-1, kind="stable")[:, :TOPK_GROUP]
    gmask = np.zeros((T, N_GROUP), bool)
    gmask[np.arange(T)[:, None], gidx] = True
    emask = np.repeat(gmask, E // N_GROUP, axis=1)           # [T, E]
    masked = np.where(emask, choice, -np.inf)
    topk_ids = np.argsort(-masked, axis=-1, kind="stable")[:, :TOP_K]
    topk_w = np.take_along_axis(scores, topk_ids, axis=1)
    topk_w = topk_w / topk_w.sum(-1, keepdims=True) * ROUTED_SCALE
    return topk_ids.astype(np.int32), topk_w


def _pack_lhs_panels(w, n_m, n_k):
    """[n_m*128, n_k*128] (indexed [M, K]) -> [n_m, 128, n_k, 128] panels
    where panel[m][p, k, j] = w[128*m + j, 128*k + p], i.e. each panel
    slice [:, k, :] is the lhsT chunk [K-sub=128, M-sub=128]."""
    a = w.reshape(n_m, P, n_k, P)          # [m, j, k, p]
    return np.ascontiguousarray(a.transpose(0, 3, 2, 1))


def _pack_rhs(xcols):
    """[C, H] token-major rows -> [128, KH, C] rhs layout:
    out[p, k, c] = xcols[c, 128*k + p]."""
    a = xcols.reshape(-1, KH, P)           # [c, k, p]
    return np.ascontiguousarray(a.transpose(2, 1, 0))


def _nchunks(c):
    out = []
    o = 0
    while o < c:
        n = min(512, c - o)
        out.append((o, n))
        o += n
    return out


def _build_program(CB, CS):
    """One SPMD Tile program shared by all 8 cores. CB/CS: routed slot
    capacities (multiples of 128; CS may be 0 to drop the small slot)."""
    nc = bacc.Bacc(None, target_bir_lowering=False)
    bf = mybir.dt.bfloat16
    f32 = mybir.dt.float32

    slot_caps = [c for c in (CB, CS) if c > 0]
    ns = len(slot_caps)

    # --- I/O ----------------------------------------------------------
    w13q = [nc.dram_tensor(f"w13q{s}", [MW, P, KH, P], bf, kind="ExternalInput")
            for s in range(ns)]
    w2q = [nc.dram_tensor(f"w2q{s}", [MH, P, KI, P], bf, kind="ExternalInput")
           for s in range(ns)]
    xgq = [nc.dram_tensor(f"xgq{s}", [P, KH, slot_caps[s]], bf, kind="ExternalInput")
           for s in range(ns)]
    wtb = [nc.dram_tensor(f"wtb{s}", [P, slot_caps[s]], f32, kind="ExternalInput")
           for s in range(ns)]
    sguq = nc.dram_tensor("sguq", [2 * KS, P, KH, P], bf, kind="ExternalInput")
    sdq = nc.dram_tensor("sdq", [MH, P, KS, P], bf, kind="ExternalInput")
    xtq = nc.dram_tensor("xtq", [P, KH, T], bf, kind="ExternalInput")

    yout = [nc.dram_tensor(f"y{s}", [MH, P, slot_caps[s]], f32, kind="ExternalOutput")
            for s in range(ns)]
    shp = nc.dram_tensor("shp", [MH, P, T], f32, kind="ExternalOutput")

    with tile.TileContext(nc) as tc:
        with (
            tc.tile_pool(name="resident", bufs=1) as res,
            tc.tile_pool(name="wpanel", bufs=4) as wpool,
            tc.tile_pool(name="hbuf", bufs=1) as hpool,
            tc.tile_pool(name="silu", bufs=4) as spool,
            tc.tile_pool(name="outbuf", bufs=3) as opool,
            tc.tile_pool(name="psum1", bufs=5, space="PSUM") as psum1,
            tc.tile_pool(name="psum2", bufs=3, space="PSUM") as psum2,
        ):
            # Resident activations
            xg_t, wt_t = [], []
            for s in range(ns):
                c = slot_caps[s]
                t = res.tile([P, KH, c], bf, name=f"xg{s}_t")
                nc.sync.dma_start(t[:], xgq[s].ap()[:])
                xg_t.append(t)
                w = res.tile([P, c], f32, name=f"wt{s}_t")
                nc.sync.dma_start(w[:], wtb[s].ap()[:])
                wt_t.append(w)
            xt_t = res.tile([P, KH, T], bf)
            nc.sync.dma_start(xt_t[:], xtq.ap()[:])
            sd_t = res.tile([P, KS, H], bf)   # resident shared-down panels
            for m in range(MH):
                nc.sync.dma_start(sd_t[:, :, m * P:(m + 1) * P], sdq.ap()[m])

            h_t = [hpool.tile([P, KI, slot_caps[s]], bf, name=f"h{s}_t", tag=f"h{s}_t")
                   for s in range(ns)]
            hs_t = hpool.tile([P, KS, T], bf)

            def gemm1_swiglu(wq_ap, rhs_t, n_pairs, pair_gap, n_k, cap, h_out):
                """h_out[:, pr, :] = silu(gu[pr]) * gu[pr + pair_gap]."""
                for pr in range(n_pairs):
                    panels, psums = [], []
                    for m in (pr, pr + pair_gap):
                        pan = wpool.tile([P, KH, P], bf, tag="wpanel1")
                        nc.sync.dma_start(pan[:, :n_k, :], wq_ap[m])
                        panels.append(pan)
                        ps = [psum1.tile([P, 512], mybir.dt.float32, tag="ps_g1",
                                         name=f"ps_g1_{pr}_{m}_{ci}")
                              for ci in range(len(_nchunks(cap)))]
                        for k in range(n_k):
                            for ci, (o, n) in enumerate(_nchunks(cap)):
                                nc.tensor.matmul(
                                    ps[ci][:, :n],
                                    lhsT=pan[:, k, :],
                                    rhs=rhs_t[:, k, o:o + n],
                                    start=(k == 0),
                                    stop=(k == n_k - 1),
                                )
                        psums.append(ps)
                    for ci, (o, n) in enumerate(_nchunks(cap)):
                        # silu(g) * u as sigmoid(g) * g * u (Silu itself is
                        # not implemented in CoreSim).
                        sg = spool.tile([P, 512], mybir.dt.float32, tag="sg")
                        nc.scalar.activation(
                            sg[:, :n], psums[0][ci][:, :n],
                            mybir.ActivationFunctionType.Sigmoid,
                        )
                        nc.vector.tensor_mul(
                            sg[:, :n], sg[:, :n], psums[0][ci][:, :n],
                        )
                        nc.vector.tensor_mul(
                            h_out[:, pr, o:o + n], sg[:, :n], psums[1][ci][:, :n],
                        )

            def gemm2(wq_ap, h_in, n_k, cap, out_dram, scale_t):
                for m in range(MH):
                    pan = wpool.tile([P, KI, P], bf, tag="wpanel2")
                    nc.sync.dma_start(pan[:, :n_k, :], wq_ap[m])
                    ps = [psum2.tile([P, 512], mybir.dt.float32, tag="ps_g2",
                                     name=f"ps_g2_{m}_{ci}")
                          for ci in range(len(_nchunks(cap)))]
                    for k in range(n_k):
                        for ci, (o, n) in enumerate(_nchunks(cap)):
                            nc.tensor.matmul(
                                ps[ci][:, :n],
                                lhsT=pan[:, k, :],
                                rhs=h_in[:, k, o:o + n],
                                start=(k == 0),
                                stop=(k == n_k - 1),
                            )
                    ot = opool.tile([P, cap], mybir.dt.float32, tag="yout")
                    for ci, (o, n) in enumerate(_nchunks(cap)):
                        if scale_t is not None:
                            nc.vector.tensor_mul(
                                ot[:, o:o + n], ps[ci][:, :n], scale_t[:, o:o + n],
                            )
                        else:
                            nc.any.tensor_copy(ot[:, o:o + n], ps[ci][:, :n])
                    nc.sync.dma_start(out_dram.ap()[m], ot[:])

            # Phase order keeps the PE fed: slot-1 GEMM1 runs while slot-0's
            # last SwiGLU drains, etc.
            for s in range(ns):
                gemm1_swiglu(w13q[s].ap(), xg_t[s], NPAIR, NPAIR, KH,
                             slot_caps[s], h_t[s])
            gemm1_swiglu(sguq.ap(), xt_t, KS, KS, KH, T, hs_t)
            for s in range(ns):
                gemm2(w2q[s].ap(), h_t[s], KI, slot_caps[s], yout[s], wt_t[s])

            # Shared GEMM2 from resident sd_t panels.
            for m in range(MH):
                ps = [psum2.tile([P, 512], mybir.dt.float32, tag="ps_g2",
                                 name=f"ps_sh_{m}_{ci}")
                      for ci in range(len(_nchunks(T)))]
                for k in range(KS):
                    for ci, (o, n) in enumerate(_nchunks(T)):
                        nc.tensor.matmul(
                            ps[ci][:, :n],
                            lhsT=sd_t[:, k, m * P:(m + 1) * P],
                            rhs=hs_t[:, k, o:o + n],
                            start=(k == 0),
                            stop=(k == KS - 1),
                        )
                ot = opool.tile([P, T], mybir.dt.float32, tag="shout")
                for ci, (o, n) in enumerate(_nchunks(T)):
                    nc.any.tensor_copy(ot[:, o:o + n], ps[ci][:, :n])
                nc.sync.dma_start(shp.ap()[m], ot[:])

    nc.compile()
    return nc


_PROGRAM_CACHE = {}


def _get_program(CB, CS):
    key = (CB, CS)
    if key not in _PROGRAM_CACHE:
        _PROGRAM_CACHE[key] = _build_program(CB, CS)
    return _PROGRAM_CACHE[key]


def _prepare(x, gate_weight, gate_bias, w13, w2, shared_gate_up, shared_down):
    """Host-side routing + packing. Returns (CB, CS, in_maps, meta)."""
    topk_ids, topk_w = _route(x, gate_weight, gate_bias)
    flat_e = topk_ids.ravel()
    flat_w = topk_w.ravel()
    flat_t = np.repeat(np.arange(T, dtype=np.int64), TOP_K)
    idx_e = [flat_t[flat_e == e] for e in range(E)]
    w_e = [flat_w[flat_e == e] for e in range(E)]
    counts = np.array([len(i) for i in idx_e])

    order = np.argsort(-counts, kind="stable")
    big, small = order[:N_CORES], order[N_CORES:]
    cmax_b = counts[big].max()
    cmax_s = counts[small].max() if len(small) else 0
    CB = max(P, int(-(-cmax_b // P)) * P)
    CS = int(-(-cmax_s // P)) * P if cmax_s > 0 else 0

    xt_pack = _pack_rhs(x.astype(BF16))                 # [128, KH, T]

    in_maps, meta = [], []
    for c in range(N_CORES):
        experts = [int(big[c])] + ([int(small[N_CORES - 1 - c])] if CS else [])
        caps = [CB] + ([CS] if CS else [])
        im = {}
        cmeta = []
        for s, (e, cap) in enumerate(zip(experts, caps)):
            idx = idx_e[e]
            n = len(idx)
            xg = np.zeros((cap, H), dtype=BF16)
            xg[:n] = x[idx].astype(BF16)
            im[f"xgq{s}"] = _pack_rhs(xg)
            wt = np.zeros((cap,), dtype=F32)
            wt[:n] = w_e[e].astype(F32)
            im[f"wtb{s}"] = np.ascontiguousarray(
                np.broadcast_to(wt[None, :], (P, cap)).astype(F32))
            im[f"w13q{s}"] = _pack_lhs_panels(w13[e].astype(BF16), MW, KH)
            # GEMM2 lhsT chunk (p, j) must be w2[e][128m+j, 128k+p] -> pass
            # w2[e] (indexed [M=H, K=I]) directly.
            im[f"w2q{s}"] = _pack_lhs_panels(w2[e].astype(BF16), MH, KI)
            cmeta.append((s, e, idx))
        # shared shard: rows [c*352, (c+1)*352) of gate and up, padded to 384
        sh = IS // N_CORES
        lo, hi = c * sh, (c + 1) * sh
        gsl = np.zeros((SSH, H), dtype=F32)
        usl = np.zeros((SSH, H), dtype=F32)
        gsl[:hi - lo] = shared_gate_up[lo:hi]
        usl[:hi - lo] = shared_gate_up[IS + lo:IS + hi]
        sgu_pad = np.concatenate([gsl, usl], 0).astype(BF16)   # [768, H]
        im["sguq"] = _pack_lhs_panels(sgu_pad, 2 * KS, KH)
        sd_sl = np.zeros((H, SSH), dtype=F32)
        sd_sl[:, :hi - lo] = shared_down[:, lo:hi]
        im["sdq"] = _pack_lhs_panels(sd_sl.astype(BF16), MH, KS)
        im["xtq"] = xt_pack
        in_maps.append(im)
        meta.append(cmeta)
    return CB, CS, in_maps, meta


def _combine(results, meta):
    out = np.zeros((H, T), dtype=F32)
    for c in range(N_CORES):
        out += results[c]["shp"].reshape(H, T)
    out = np.ascontiguousarray(out.T)                   # [T, H]
    for c in range(N_CORES):
        r = results[c]
        for (s, e, idx) in meta[c]:
            n = len(idx)
            if n:
                y = r[f"y{s}"].reshape(H, -1)           # [H, cap]
                out[idx] += y[:, :n].T
    return out


def kernel(hidden_states, gate_weight, gate_bias, w13, w2,
           shared_gate_up, shared_down):
    x = np.asarray(hidden_states, dtype=F32)
    gate_weight = np.asarray(gate_weight, dtype=F32)
    gate_bias = np.asarray(gate_bias, dtype=F32)
    w13 = np.asarray(w13, dtype=F32)
    w2 = np.asarray(w2, dtype=F32)
    shared_gate_up = np.asarray(shared_gate_up, dtype=F32)
    shared_down = np.asarray(shared_down, dtype=F32)

    CB, CS, in_maps, meta = _prepare(
        x, gate_weight, gate_bias, w13, w2, shared_gate_up, shared_down)
    nc = _get_program(CB, CS)
    res = run_bass_kernel_spmd(nc, in_maps, core_ids=list(range(N_CORES)))
    return _combine(res.results, meta)


# revision 12
# speedup vs baseline: 226.0822x; 226.0822x over previous
"""DeepseekMoE (moe_routing) Trainium2 kernel.

Strategy (8 NeuronCores, single SPMD program):
  - Routing (grouped top-k; tiny T x H @ H x E) runs on host in numpy.
  - Routed experts are expert-parallel: each core owns 2 experts (one
    "big" slot, one "small" slot; capacities sized at call time from the
    actual per-expert token counts, so the compiled shapes adapt to the
    data). Tokens for each expert are gathered host-side into a
    transposed [H, C] activation block per slot; the device runs
    grouped GEMM1 -> SwiGLU -> GEMM2 per slot with the top-k combine
    weight folded into the GEMM2 PSUM evict.
  - Shared expert MLP is tensor-parallel over the 8 cores along the
    intermediate dim (2816 -> 8 x 352, zero-padded to 8 x 384).
  - All matmul operands are bf16 (cast host-side), accumulation f32.
  - Device returns per-slot y^T [H, C] plus the shared partial [H, T];
    host sums partials and scatter-adds slot outputs.
"""

import numpy as np
import ml_dtypes

import concourse.mybir as mybir
import concourse.tile as tile
from concourse import bacc
from concourse.bass_utils import run_bass_kernel_spmd

BF16 = ml_dtypes.bfloat16
F32 = np.float32

# Problem shapes (fixed by the spec).
T, H, E, I = 1024, 2048, 16, 1408
I2 = 2 * I                      # 2816 (w13 rows per expert)
IS = 2 * I                      # shared intermediate (n_shared=2 -> 2816)
SSH = 384                       # per-core shared shard (2816 padded to 3072 = 8*384)
TOP_K, N_GROUP, TOPK_GROUP = 4, 4, 2
ROUTED_SCALE = 2.5
N_CORES = 8
P = 128
KH = H // P                     # 16 K-subtiles over H
KI = I // P                     # 11 K-subtiles over I
MW = I2 // P                    # 22 M-panels over 2I
MH = H // P                     # 16 M-panels over H
NPAIR = I // P                  # 11 (g,u) SwiGLU pairs per slot
KS = SSH // P                   # 3 K-subtiles over shared shard


def _sigmoid(x):
    return 1.0 / (1.0 + np.exp(-x))


def _route(x, gate_weight, gate_bias):
    """Numpy port of reference._grouped_topk (float64 internally)."""
    logits = x.astype(np.float64) @ gate_weight.astype(np.float64).T
    scores = _sigmoid(logits)
    choice = scores + gate_bias.astype(np.float64)[None, :]
    g = choice.reshape(T, N_GROUP, E // N_GROUP)
    top2sum = np.sort(g, axis=-1)[..., -2:].sum(-1)          # [T, NG]
    gidx = np.argsort(-top2sum, axis=-1, kind="stable")[:, :TOPK_GROUP]
    gmask = np.zeros((T, N_GROUP), bool)
    gmask[np.arange(T)[:, None], gidx] = True
    emask = np.repeat(gmask, E // N_GROUP, axis=1)           # [T, E]
    masked = np.where(emask, choice, -np.inf)
    topk_ids = np.argsort(-masked, axis=-1, kind="stable")[:, :TOP_K]
    topk_w = np.take_along_axis(scores, topk_ids, axis=1)
    topk_w = topk_w / topk_w.sum(-1, keepdims=True) * ROUTED_SCALE
    return topk_ids.astype(np.int32), topk_w


def _pack_lhs_panels(w, n_m, n_k):
    """[n_m*128, n_k*128] (indexed [M, K]) -> [n_m, 128, n_k, 128] panels
    where panel[m][p, k, j] = w[128*m + j, 128*k + p], i.e. each panel
    slice [:, k, :] is the lhsT chunk [K-sub=128, M-sub=128]."""
    a = w.reshape(n_m, P, n_k, P)          # [m, j, k, p]
    return np.ascontiguousarray(a.transpose(0, 3, 2, 1))


def _pack_rhs(xcols):
    """[C, H] token-major rows -> [128, KH, C] rhs layout:
    out[p, k, c] = xcols[c, 128*k + p]."""
    a = xcols.reshape(-1, KH, P)           # [c, k, p]
    return np.ascontiguousarray(a.transpose(2, 1, 0))


def _nchunks(c):
    out = []
    o = 0
    while o < c:
        n = min(512, c - o)
        out.append((o, n))
        o += n
    return out


def _build_program(CB, CS, reps=1):
    """One SPMD Tile program shared by all 8 cores. CB/CS: routed slot
    capacities (CS may be 0 to drop the small slot). reps>1 wraps the
    compute in a hardware loop (timing amplification only)."""
    nc = bacc.Bacc(None, target_bir_lowering=False)
    bf = mybir.dt.bfloat16
    f32 = mybir.dt.float32

    slot_caps = [c for c in (CB, CS) if c > 0]
    ns = len(slot_caps)

    # --- I/O ----------------------------------------------------------
    w13q = [nc.dram_tensor(f"w13q{s}", [MW, P, KH, P], bf, kind="ExternalInput")
            for s in range(ns)]
    w2q = [nc.dram_tensor(f"w2q{s}", [MH, P, KI, P], bf, kind="ExternalInput")
           for s in range(ns)]
    xgq = [nc.dram_tensor(f"xgq{s}", [P, KH, slot_caps[s]], bf, kind="ExternalInput")
           for s in range(ns)]
    wtb = [nc.dram_tensor(f"wtb{s}", [P, slot_caps[s]], f32, kind="ExternalInput")
           for s in range(ns)]
    sguq = nc.dram_tensor("sguq", [2 * KS, P, KH, P], bf, kind="ExternalInput")
    sdq = nc.dram_tensor("sdq", [MH, P, KS, P], bf, kind="ExternalInput")
    xtq = nc.dram_tensor("xtq", [P, KH, T], bf, kind="ExternalInput")

    yout = [nc.dram_tensor(f"y{s}", [MH, P, slot_caps[s]], f32, kind="ExternalOutput")
            for s in range(ns)]
    shp = nc.dram_tensor("shp", [MH, P, T], f32, kind="ExternalOutput")

    with tile.TileContext(nc) as tc:
        with (
            tc.tile_pool(name="resident", bufs=1) as res,
            tc.tile_pool(name="wpanel", bufs=8) as wpool,
            tc.tile_pool(name="hbuf", bufs=1) as hpool,
            tc.tile_pool(name="silu", bufs=4) as spool,
            tc.tile_pool(name="outbuf", bufs=4) as opool,
            tc.tile_pool(name="psum", bufs=8, space="PSUM") as psum1,
        ):
            # Resident activations
            xg_t, wt_t = [], []
            for s in range(ns):
                c = slot_caps[s]
                t = res.tile([P, KH, c], bf, name=f"xg{s}_t")
                nc.sync.dma_start(t[:], xgq[s].ap()[:])
                xg_t.append(t)
                w = res.tile([P, c], f32, name=f"wt{s}_t")
                nc.sync.dma_start(w[:], wtb[s].ap()[:])
                wt_t.append(w)
            xt_t = res.tile([P, KH, T], bf)
            nc.sync.dma_start(xt_t[:], xtq.ap()[:])
            sd_t = res.tile([P, KS, H], bf)   # resident shared-down panels
            for m in range(MH):
                nc.sync.dma_start(sd_t[:, :, m * P:(m + 1) * P], sdq.ap()[m])

            h_t = [hpool.tile([P, KI, slot_caps[s]], bf, name=f"h{s}_t", tag=f"h{s}_t")
                   for s in range(ns)]
            hs_t = hpool.tile([P, KS, T], bf)

            def gemm1_pair(wq_ap, rhs_t, pr, pair_gap, n_k, cap, h_out):
                """h_out[:, pr, :] = silu(gu[pr]) * gu[pr + pair_gap]."""
                if True:
                    panels, psums = [], []
                    for m in (pr, pr + pair_gap):
                        pan = wpool.tile([P, KH, P], bf, tag="wpanel1")
                        nc.sync.dma_start(pan[:, :n_k, :], wq_ap[m])
                        panels.append(pan)
                        ps = [psum1.tile([P, 512], mybir.dt.float32, tag="ps",
                                         name=f"ps_g1_{pr}_{m}_{ci}")
                              for ci in range(len(_nchunks(cap)))]
                        for k in range(n_k):
                            for ci, (o, n) in enumerate(_nchunks(cap)):
                                nc.tensor.matmul(
                                    ps[ci][:, :n],
                                    lhsT=pan[:, k, :],
                                    rhs=rhs_t[:, k, o:o + n],
                                    start=(k == 0),
                                    stop=(k == n_k - 1),
                                )
                        psums.append(ps)
                    for ci, (o, n) in enumerate(_nchunks(cap)):
                        # silu(g) * u as sigmoid(g) * g * u (Silu itself is
                        # not implemented in CoreSim).
                        sg = spool.tile([P, 512], mybir.dt.float32, tag="sg")
                        nc.scalar.activation(
                            sg[:, :n], psums[0][ci][:, :n],
                            mybir.ActivationFunctionType.Sigmoid,
                        )
                        nc.vector.tensor_mul(
                            sg[:, :n], sg[:, :n], psums[0][ci][:, :n],
                        )
                        nc.vector.tensor_mul(
                            h_out[:, pr, o:o + n], sg[:, :n], psums[1][ci][:, :n],
                        )

            def gemm2_mtile(wq_ap, h_in, n_k, cap, out_dram, scale_t, m):
                if True:
                    pan = wpool.tile([P, KI, P], bf, tag="wpanel2")
                    nc.sync.dma_start(pan[:, :n_k, :], wq_ap[m])
                    ps = [psum1.tile([P, 512], mybir.dt.float32, tag="ps",
                                     name=f"ps_g2_{m}_{ci}")
                          for ci in range(len(_nchunks(cap)))]
                    for k in range(n_k):
                        for ci, (o, n) in enumerate(_nchunks(cap)):
                            nc.tensor.matmul(
                                ps[ci][:, :n],
                                lhsT=pan[:, k, :],
                                rhs=h_in[:, k, o:o + n],
                                start=(k == 0),
                                stop=(k == n_k - 1),
                            )
                    ot = opool.tile([P, cap], mybir.dt.float32, tag="yout")
                    for ci, (o, n) in enumerate(_nchunks(cap)):
                        if scale_t is not None:
                            nc.vector.tensor_mul(
                                ot[:, o:o + n], ps[ci][:, :n], scale_t[:, o:o + n],
                            )
                        else:
                            nc.any.tensor_copy(ot[:, o:o + n], ps[ci][:, :n])
                    nc.sync.dma_start(out_dram.ap()[m], ot[:])

            def body():
                # GEMM1: interleave slot0 (long-N), slot1 (short-N,
                # LDW-bound) and shared streams pair-by-pair so short-N
                # weight loads hide under long-N matmul streaming.
                for pr in range(NPAIR):
                    for s in range(ns):
                        gemm1_pair(w13q[s].ap(), xg_t[s], pr, NPAIR, KH,
                                   slot_caps[s], h_t[s])
                    if pr < KS:
                        gemm1_pair(sguq.ap(), xt_t, pr, KS, KH, T, hs_t)
                # Shared GEMM2 first: its 8 MB output drains while the
                # routed GEMM2 phases compute.
                for m in range(MH):
                    ps = [psum1.tile([P, 512], mybir.dt.float32, tag="ps",
                                     name=f"ps_sh_{m}_{ci}")
                          for ci in range(len(_nchunks(T)))]
                    for k in range(KS):
                        for ci, (o, n) in enumerate(_nchunks(T)):
                            nc.tensor.matmul(
                                ps[ci][:, :n],
                                lhsT=sd_t[:, k, m * P:(m + 1) * P],
                                rhs=hs_t[:, k, o:o + n],
                                start=(k == 0),
                                stop=(k == KS - 1),
                            )
                    ot = opool.tile([P, T], mybir.dt.float32, tag="shout")
                    for ci, (o, n) in enumerate(_nchunks(T)):
                        nc.any.tensor_copy(ot[:, o:o + n], ps[ci][:, :n])
                    nc.sync.dma_start(shp.ap()[m], ot[:])

                for m in range(MH):
                    for s in range(ns):
                        gemm2_mtile(w2q[s].ap(), h_t[s], KI, slot_caps[s],
                                    yout[s], wt_t[s], m)

            if reps == 1:
                body()
            else:
                with tc.For_i(0, reps, 1):
                    body()

    nc.compile()
    return nc


_PROGRAM_CACHE = {}


def _get_program(CB, CS):
    key = (CB, CS)
    if key not in _PROGRAM_CACHE:
        _PROGRAM_CACHE[key] = _build_program(CB, CS)
    return _PROGRAM_CACHE[key]


def _prepare(x, gate_weight, gate_bias, w13, w2, shared_gate_up, shared_down):
    """Host-side routing + packing. Returns (CB, CS, in_maps, meta)."""
    topk_ids, topk_w = _route(x, gate_weight, gate_bias)
    flat_e = topk_ids.ravel()
    flat_w = topk_w.ravel()
    flat_t = np.repeat(np.arange(T, dtype=np.int64), TOP_K)
    idx_e = [flat_t[flat_e == e] for e in range(E)]
    w_e = [flat_w[flat_e == e] for e in range(E)]
    counts = np.array([len(i) for i in idx_e])

    order = np.argsort(-counts, kind="stable")
    big, small = order[:N_CORES], order[N_CORES:]
    cmax_b = counts[big].max()
    cmax_s = counts[small].max() if len(small) else 0
    # round capacities to a multiple of 8 (DMA alignment); slot compute
    # scales with capacity so keep it tight.
    CB = max(8, int(-(-cmax_b // 8)) * 8)
    CS = int(-(-cmax_s // 8)) * 8 if cmax_s > 0 else 0

    xt_pack = _pack_rhs(x.astype(BF16))                 # [128, KH, T]

    in_maps, meta = [], []
    for c in range(N_CORES):
        experts = [int(big[c])] + ([int(small[N_CORES - 1 - c])] if CS else [])
        caps = [CB] + ([CS] if CS else [])
        im = {}
        cmeta = []
        for s, (e, cap) in enumerate(zip(experts, caps)):
            idx = idx_e[e]
            n = len(idx)
            xg = np.zeros((cap, H), dtype=BF16)
            xg[:n] = x[idx].astype(BF16)
            im[f"xgq{s}"] = _pack_rhs(xg)
            wt = np.zeros((cap,), dtype=F32)
            wt[:n] = w_e[e].astype(F32)
            im[f"wtb{s}"] = np.ascontiguousarray(
                np.broadcast_to(wt[None, :], (P, cap)).astype(F32))
            im[f"w13q{s}"] = _pack_lhs_panels(w13[e].astype(BF16), MW, KH)
            # GEMM2 lhsT chunk (p, j) must be w2[e][128m+j, 128k+p] -> pass
            # w2[e] (indexed [M=H, K=I]) directly.
            im[f"w2q{s}"] = _pack_lhs_panels(w2[e].astype(BF16), MH, KI)
            cmeta.append((s, e, idx))
        # shared shard: rows [c*352, (c+1)*352) of gate and up, padded to 384
        sh = IS // N_CORES
        lo, hi = c * sh, (c + 1) * sh
        gsl = np.zeros((SSH, H), dtype=F32)
        usl = np.zeros((SSH, H), dtype=F32)
        gsl[:hi - lo] = shared_gate_up[lo:hi]
        usl[:hi - lo] = shared_gate_up[IS + lo:IS + hi]
        sgu_pad = np.concatenate([gsl, usl], 0).astype(BF16)   # [768, H]
        im["sguq"] = _pack_lhs_panels(sgu_pad, 2 * KS, KH)
        sd_sl = np.zeros((H, SSH), dtype=F32)
        sd_sl[:, :hi - lo] = shared_down[:, lo:hi]
        im["sdq"] = _pack_lhs_panels(sd_sl.astype(BF16), MH, KS)
        im["xtq"] = xt_pack
        in_maps.append(im)
        meta.append(cmeta)
    return CB, CS, in_maps, meta


def _combine(results, meta):
    out = np.zeros((H, T), dtype=F32)
    for c in range(N_CORES):
        out += results[c]["shp"].reshape(H, T)
    out = np.ascontiguousarray(out.T)                   # [T, H]
    for c in range(N_CORES):
        r = results[c]
        for (s, e, idx) in meta[c]:
            n = len(idx)
            if n:
                y = r[f"y{s}"].reshape(H, -1)           # [H, cap]
                out[idx] += y[:, :n].T
    return out


def kernel(hidden_states, gate_weight, gate_bias, w13, w2,
           shared_gate_up, shared_down):
    x = np.asarray(hidden_states, dtype=F32)
    gate_weight = np.asarray(gate_weight, dtype=F32)
    gate_bias = np.asarray(gate_bias, dtype=F32)
    w13 = np.asarray(w13, dtype=F32)
    w2 = np.asarray(w2, dtype=F32)
    shared_gate_up = np.asarray(shared_gate_up, dtype=F32)
    shared_down = np.asarray(shared_down, dtype=F32)

    CB, CS, in_maps, meta = _prepare(
        x, gate_weight, gate_bias, w13, w2, shared_gate_up, shared_down)
    nc = _get_program(CB, CS)
    res = run_bass_kernel_spmd(nc, in_maps, core_ids=list(range(N_CORES)))
    return _combine(res.results, meta)
